# revision 13
# baseline (speedup 1.0000x reference)
"""Trainium2 Bass kernel for nn_DAInsHead (moe_routing).

Per-row hard-routed 3-layer MLP: rows with levels[i]==l get
    out[i] = W3[l].T @ relu(W2[l].T @ relu(W1[l].T @ x[i] + b1[l]) + b2[l]) + b3[l]

Strategy (vs the reference's dense 4x-redundant masked compute):
  * Host: stable-sort rows by level, deal each level's rows evenly to the 8
    cores, pad each (core, level) segment to a shared per-level capacity
    (multiple of 8, decomposed into row tiles of 256..512), and transpose to
    feature-major xT [D, R_core] so the device needs no on-chip transposes.
  * Host folds W3 into layer 2, turning relu+W3-matvec into plain relu
    evictions plus an unweighted reduction: every folded column carries
    |w3|, and the sign of w3 is realized by ADD vs SUBTRACT chunk groups in
    a DVE tree-sum plus a per-partition +-1 vector sigma in the final
    128-partition matvec.  Hidden units are permuted to slots (mc, p) such
    that sign(w3) == sigma[p] * (+1 if mc < a else -1) -- a counting
    argument needs an even number of w3>0 units; odd counts drop the single
    smallest-|w3| unit (~1e-5 relative noise, far under both the f32r noise
    floor and the 2e-2 gate).  Levels with no (a, sigma) solution fall back
    to sign-sorted pure chunks with one mixed boundary chunk reduced by an
    extra +-1-mask matvec.
  * Device (identical SPMD program on 8 cores): for each level, keep that
    level's W1/W2' resident in SBUF and stream row tiles of ~512: L1/L2 are
    K=8-chunk accumulated 128x128x512 bf16 matmuls (bf16 weights remove the
    ~32-cycle/matmul weight-swap stall f32r pays: 213ns/512-row matmul vs
    227ns, and halve weight+x DMA) with relu+bias evictions on the scalar
    (ACT) engine; the evicted
    L2 chunks are tree-summed on DVE (interleaved per half so the adds hide
    under the other half's matmuls) and one sigma-matvec on the PE does the
    final 128-partition reduction (1/8th the PE cost of a per-chunk
    matvec).  The matvec for tile t issues after tile t+1's L1 matmuls so
    its DVE dependencies never stall the PE.  Level-0 DMAs are issued in
    exact consumption order (x chunk / W1 chunk interleaved) to minimize
    the startup stall while the 8.4MB weight stream lands.
  * Host: scatter per-core outputs back to original row order.

Precision: x/W1/h1/W2 are bf16 (each ~0.2% quantization noise); the
reduction path (h2 evictions, tree-sum, sigma-matvec, PSUM) stays
f32r/f32.  Measured scale-relative max error 3.4e-3 vs the 2e-2 gate.
bf16 and f32r run the PE at the same 1 cycle/row, so the cast costs no
matmul throughput -- it buys DMA bytes and the weight-swap stall.
"""
import os
import sys

sys.path.insert(0, "/opt/trn_rl_repo")

import ml_dtypes
import numpy as np

import concourse.bacc as bacc
import concourse.mybir as mybir
import concourse.tile as tile
from concourse.bass_utils import run_bass_kernel_spmd

F32 = mybir.dt.float32
F32R = mybir.dt.float32r
BF16 = mybir.dt.bfloat16
BF16_NP = ml_dtypes.bfloat16
ADD = mybir.AluOpType.add
MAX = mybir.AluOpType.max
MIN = mybir.AluOpType.min
SUB = mybir.AluOpType.subtract
RELU = mybir.ActivationFunctionType.Relu

NC = 8          # cores
L = 4           # levels
D = 1024        # in features
H = 1024        # hidden
KC = D // 128   # contraction chunks
MC = H // 128   # output chunks

LAST_RESULTS = None       # BassKernelResults of the most recent run (for test.py)
_PROGRAM_CACHE = {}


def _row_tiles(c):
    """Split a per-level capacity (multiple of 8, >=256) into row-tile sizes
    in [256, 512] (f32r matmuls need a moving dim >=256 for full rate; a
    PSUM bank holds at most 512 fp32 per partition)."""
    if c <= 512:
        return [c]
    q, r = divmod(c, 512)
    tiles = [512] * q
    if r == 0:
        pass
    elif r >= 256:
        tiles.append(r)
    else:
        # split the last 512+r rows into two tiles >= 256
        tiles[-1:] = [256 + r, 256]
    return tiles


def _fold_level_sigma(w3v, W2l, b2l):
    """Sigma-scheme fold of w3 into W2/b2. Returns (W2pp, b2pp, sigma, a)
    or None when no (a, q) sign assignment exists."""
    npos = int((w3v > 0).sum())
    pos_units = list(np.where(w3v > 0)[0])
    neg_units = list(np.where(w3v <= 0)[0])
    cands = [(npos, None)] if npos % 2 == 0 else [(npos - 1, "pos"), (npos + 1, "neg")]
    for C, drop in cands:
        sol = None
        for a in range(MC + 1):
            for q in range(129):
                if a * q + (MC - a) * (128 - q) == C:
                    sol = (a, q)
                    break
            if sol:
                break
        if sol is None:
            continue
        a, q = sol
        pu, nu = list(pos_units), list(neg_units)
        if drop == "pos":
            pu.remove(min(pu, key=lambda u: abs(w3v[u])))
        elif drop == "neg":
            nu.remove(min(nu, key=lambda u: abs(w3v[u])))
        W2pp = np.zeros_like(W2l)
        b2pp = np.zeros_like(b2l)
        sigma = np.where(np.arange(128) < q, 1.0, -1.0).astype(np.float32)
        pos_slots = [(mc, p) for mc in range(MC) for p in range(128) if (mc < a) == (p < q)]
        neg_slots = [(mc, p) for mc in range(MC) for p in range(128) if (mc < a) != (p < q)]
        for slots, units in ((pos_slots, pu), (neg_slots, nu)):
            for (mc, p), u in zip(slots, units):
                j = mc * 128 + p
                f = abs(w3v[u])
                W2pp[:, j] = W2l[:, u] * f
                b2pp[j] = b2l[u] * f
        if a == 0:
            sigma = -sigma  # all chunks subtract; absorb the minus into sigma
        return W2pp, b2pp, sigma, a
    return None


def _build_program(caps, modes):
    """Build + compile the SPMD program.

    caps: per-level row capacities.
    modes: per-level ('s', a) for the sigma scheme (chunks < a max-evicted,
    rest min) or ('m', cb) for the masked-boundary fallback (chunk cb mixed).
    """
    r_core = sum(caps)
    nc = bacc.Bacc("TRN2", target_bir_lowering=False, debug=False, num_devices=NC)
    xT = nc.dram_tensor("xT", [D, r_core], BF16, kind="ExternalInput")
    W1 = nc.dram_tensor("W1", [L, D, H], BF16, kind="ExternalInput")
    W2 = nc.dram_tensor("W2", [L, H, H], BF16, kind="ExternalInput")  # w3-folded
    b1 = nc.dram_tensor("b1", [L, H], F32, kind="ExternalInput")
    b2 = nc.dram_tensor("b2", [L, H], F32, kind="ExternalInput")      # w3-folded
    b3 = nc.dram_tensor("b3", [L, 1], F32, kind="ExternalInput")
    msk = nc.dram_tensor("msk", [L, 128, 3], BF16, kind="ExternalInput")
    out = nc.dram_tensor("out", [1, r_core], F32, kind="ExternalOutput")

    xT_r = xT.rearrange("(kc p) r -> p kc r", p=128)  # [128, KC, r_core]

    with tile.TileContext(nc) as tc:
        with (
            tc.tile_pool(name="wpool", bufs=2) as wpool,
            tc.tile_pool(name="bpool", bufs=2) as bpool,
            tc.tile_pool(name="xpool", bufs=2) as xpool,
            tc.tile_pool(name="hpool", bufs=1) as hpool,
            tc.tile_pool(name="opool", bufs=3) as opool,
            tc.tile_pool(name="ps", bufs=7, space="PSUM") as ps,
            tc.tile_pool(name="ps3", bufs=1, space="PSUM") as ps3,
        ):
            # Deferred final reduction: the matvecs for tile t issue after
            # tile t+1's L1 matmuls so the PE never waits on the DVE
            # tree-sum feeding them.
            pending = []

            def flush_l3():
                if not pending:
                    return
                mms, b3t_, off_, rt_ = pending.pop()
                acc3 = ps3.tile([1, rt_], F32)
                for i, (w_ap, rhs_ap) in enumerate(mms):
                    nc.tensor.matmul(acc3[:], w_ap, rhs_ap,
                                     start=(i == 0), stop=(i == len(mms) - 1))
                o_t = opool.tile([1, rt_], F32, tag="o")
                nc.vector.tensor_scalar(o_t[:], acc3[:], b3t_[:], None, ADD)
                nc.gpsimd.dma_start(out[:, off_:off_ + rt_], o_t[:])

            off = 0
            last_lvl = max(l for l in range(L) if caps[l])
            for lvl in range(L):
                cap = caps[lvl]
                if cap == 0:
                    continue
                mode = modes[lvl]
                sigma_mode = mode[0] == "s"
                a_or_cb = mode[1]
                tiles_l = _row_tiles(cap)
                if lvl == 0:
                    tiles_l = sorted(tiles_l)
                # For level 0, issue the first row-tile's first x chunk and
                # the first W1 chunk before everything else: the very first
                # matmul only needs those ~640KB, so the PE starts ~2us
                # after DMA flow begins instead of waiting behind the full
                # x tile + weight stream.
                pre_x = []
                w1k = [None] * KC
                if lvl == 0:
                    rt0 = tiles_l[0]
                    px = xpool.tile([128, KC, rt0], BF16, tag="x")
                    pre_x.append(px)
                    nc.sync.dma_start(px[:, 0, :], xT_r[:, 0, 0:rt0])
                    t1 = wpool.tile([128, H], BF16, tag="w1k0")
                    nc.sync.dma_start(t1[:], W1[lvl][0:128, :])
                    w1k[0] = t1
                # Tiny bias/mask tiles next so evictions never wait behind
                # the 8MB of W1/W2 weight DMA.
                b1t = bpool.tile([128, MC], F32, tag="b1")
                nc.sync.dma_start(b1t[:], b1[lvl].rearrange("(mc p) -> p mc", p=128))
                b2t = bpool.tile([128, MC], F32, tag="b2")
                nc.sync.dma_start(b2t[:], b2[lvl].rearrange("(mc p) -> p mc", p=128))
                b3t = bpool.tile([1, 1], F32, tag="b3")
                nc.sync.dma_start(b3t[:], b3[lvl:lvl + 1, :])
                # mask tile: col0/1 = boundary-chunk max/min masks (fallback
                # mode), col2 = the per-partition reduction vector (sigma,
                # or all-ones in fallback mode)
                mt = bpool.tile([128, 3], BF16, tag="mask")
                nc.sync.dma_start(mt[:], msk[lvl])
                # Per-kc weight tiles so the first matmuls only wait on the
                # first 512KB of weight DMA, and level l+1 prefetch
                # double-buffers against level l (bufs=2 per tag).  For
                # level 0 the x chunks interleave with the W1 chunks in
                # exactly the order the L1 kc-loop consumes them, so the PE
                # rides the DMA stream with minimal stall.
                for kc in range(KC):
                    if lvl == 0 and kc > 0:
                        nc.sync.dma_start(pre_x[0][:, kc, :], xT_r[:, kc, 0:rt0])
                    if w1k[kc] is None:
                        t1 = wpool.tile([128, H], BF16, tag=f"w1k{kc}")
                        nc.sync.dma_start(t1[:], W1[lvl][kc * 128:(kc + 1) * 128, :])
                        w1k[kc] = t1
                if lvl == 0 and len(tiles_l) > 1:
                    # prefetch tile 1's x ahead of the W2 stream (L2 weights
                    # aren't needed until ~15us in; tile 1's x is)
                    rt1 = tiles_l[1]
                    px1 = xpool.tile([128, KC, rt1], BF16, tag="x")
                    nc.sync.dma_start(px1[:], xT_r[:, :, tiles_l[0]:tiles_l[0] + rt1])
                    pre_x.append(px1)
                w2k = []
                for kc in range(KC):
                    t2 = wpool.tile([128, H], BF16, tag=f"w2k{kc}")
                    nc.sync.dma_start(t2[:], W2[lvl][kc * 128:(kc + 1) * 128, :])
                    w2k.append(t2)

                for ti, rt in enumerate(tiles_l):
                    # on the run's very last tile, complete each mc's 8-kc
                    # accumulation chain before the next (mc-outer) and
                    # evict immediately: the eviction/tree chain overlaps
                    # the remaining matmuls instead of being fully exposed
                    # after them (nothing follows the last tile)
                    is_last = lvl == last_lvl and ti == len(tiles_l) - 1
                    if lvl == 0 and ti < len(pre_x):
                        x_t = pre_x[ti]
                    else:
                        x_t = xpool.tile([128, KC, rt], BF16, tag="x")
                        nc.sync.dma_start(x_t[:], xT_r[:, :, off:off + rt])

                    # L1 runs kc-outer in two 4-bank halves: the first matmul
                    # only depends on w1k[0] + x_t, so the PE ramps with the
                    # weight DMA stream instead of waiting for all of W1.
                    h1 = hpool.tile([128, MC, rt], BF16, tag="h1")
                    for half in range(2):
                        mcs = range(4 * half, 4 * half + 4)
                        accs = {mc: ps.tile([128, rt], F32, tag="acc", name="acc")
                                for mc in mcs}
                        if is_last:
                            for mc in mcs:
                                for kc in range(KC):
                                    nc.tensor.matmul(
                                        accs[mc][:], w1k[kc][:, mc * 128:(mc + 1) * 128],
                                        x_t[:, kc, :], start=(kc == 0), stop=(kc == KC - 1))
                                nc.scalar.activation(
                                    h1[:, mc, :], accs[mc][:], RELU,
                                    bias=b1t[:, mc:mc + 1], scale=1.0)
                        else:
                            for kc in range(KC):
                                for mc in mcs:
                                    nc.tensor.matmul(
                                        accs[mc][:], w1k[kc][:, mc * 128:(mc + 1) * 128],
                                        x_t[:, kc, :], start=(kc == 0), stop=(kc == KC - 1))
                            for mc in mcs:
                                # L1 relu eviction on the (otherwise idle)
                                # scalar engine, keeping DVE capacity for the
                                # L2 evictions + tree-sum
                                nc.scalar.activation(
                                    h1[:, mc, :], accs[mc][:], RELU,
                                    bias=b1t[:, mc:mc + 1], scale=1.0)

                    # previous tile's final reduction: its PE dependencies
                    # (DVE tree-sum) completed long ago; slots in here
                    # without stalling the next L2 matmuls.
                    flush_l3()

                    # All folded columns carry |w3|, so every chunk evicts
                    # with a plain relu; the chunk's sign enters via ADD vs
                    # SUBTRACT in the tree (and the +-1 boundary mask in
                    # fallback mode).
                    h2 = hpool.tile([128, MC, rt], BF16, tag="h2")
                    posw = []
                    negw = []
                    for half in range(2):
                        mcs = range(4 * half, 4 * half + 4)
                        accs = {mc: ps.tile([128, rt], F32, tag="acc", name="acc")
                                for mc in mcs}
                        if is_last:
                            for mc in mcs:
                                for kc in range(MC):
                                    nc.tensor.matmul(
                                        accs[mc][:], w2k[kc][:, mc * 128:(mc + 1) * 128],
                                        h1[:, kc, :], start=(kc == 0), stop=(kc == MC - 1))
                                nc.scalar.activation(
                                    h2[:, mc, :], accs[mc][:], RELU,
                                    bias=b2t[:, mc:mc + 1], scale=1.0)
                        else:
                            for kc in range(MC):
                                for mc in mcs:
                                    nc.tensor.matmul(
                                        accs[mc][:], w2k[kc][:, mc * 128:(mc + 1) * 128],
                                        h1[:, kc, :], start=(kc == 0), stop=(kc == MC - 1))
                            for mc in mcs:
                                nc.scalar.activation(
                                    h2[:, mc, :], accs[mc][:], RELU,
                                    bias=b2t[:, mc:mc + 1], scale=1.0)
                        grp_pos = []
                        grp_neg = []
                        for mc in mcs:
                            if sigma_mode:
                                (grp_pos if mc < a_or_cb else grp_neg).append(mc)
                            elif mc != a_or_cb:
                                (grp_pos if mc < a_or_cb else grp_neg).append(mc)
                            # fallback boundary chunk joins via the +-1 mask
                            # matvec instead of the tree
                        # per-half tree-sum: these adds hide under the other
                        # half's / next tile's matmuls
                        for grp, acc_list in ((grp_pos, posw), (grp_neg, negw)):
                            while len(grp) > 1:
                                nxt = []
                                for i in range(0, len(grp) - 1, 2):
                                    nc.vector.tensor_tensor(
                                        h2[:, grp[i], :], h2[:, grp[i], :],
                                        h2[:, grp[i + 1], :], ADD)
                                    nxt.append(grp[i])
                                if len(grp) % 2:
                                    nxt.append(grp[-1])
                                grp = nxt
                            if grp:
                                acc_list.append(grp[0])
                    for lst in (posw, negw):
                        if len(lst) > 1:
                            nc.vector.tensor_tensor(
                                h2[:, lst[0], :], h2[:, lst[0], :],
                                h2[:, lst[1], :], ADD)
                            del lst[1]
                    if posw and negw:
                        nc.vector.tensor_tensor(
                            h2[:, posw[0], :], h2[:, posw[0], :],
                            h2[:, negw[0], :], SUB)
                        win = posw[0]
                    else:
                        # host flipped the sign of the reduction vector when
                        # only subtract-chunks exist
                        win = (posw or negw)[0]
                    mms = [(mt[:, 2:3], h2[:, win, :])]
                    if not sigma_mode:
                        mms.append((mt[:, 0:1], h2[:, a_or_cb, :]))
                    pending.append((mms, b3t, off, rt))
                    off += rt
            flush_l3()
    nc.compile()
    return nc


def kernel(x, levels, W1, b1, W2, b2, W3, b3):
    global LAST_RESULTS
    x = np.ascontiguousarray(np.asarray(x, dtype=np.float32))
    levels = np.asarray(levels)
    n = x.shape[0]

    # --- host-side routing: sort rows by level, deal evenly to cores ---
    order = np.argsort(levels, kind="stable")
    counts = np.bincount(np.asarray(levels, dtype=np.int64), minlength=L)[:L]

    # per-level capacity shared by all cores: ceil(max per-core count / 8)*8,
    # min 256 (row tiles below 256 lose f32r full rate)
    caps = []
    for lvl in range(L):
        per_core_max = -(-int(counts[lvl]) // NC)
        caps.append(max(-(-per_core_max // 8) * 8, 256) if per_core_max else 0)
    r_core = sum(caps)

    # --- fold W3 into layer 2 ---
    W1f = np.asarray(W1, dtype=np.float32)
    W2f = np.asarray(W2, dtype=np.float32)
    W3f = np.asarray(W3, dtype=np.float32)
    b1f = np.asarray(b1, dtype=np.float32)
    b2f = np.asarray(b2, dtype=np.float32)
    b3f = np.asarray(b3, dtype=np.float32)
    W2p = np.empty_like(W2f)
    b2p = np.empty_like(b2f)
    mskf = np.zeros((L, 128, 3), dtype=np.float32)
    modes = []
    for lvl in range(L):
        w3v = W3f[lvl, :, 0]
        folded = _fold_level_sigma(w3v, W2f[lvl], b2f[lvl])
        if folded is not None:
            W2p[lvl], b2p[lvl], sigma, a = folded
            mskf[lvl, :, 2] = sigma
            modes.append(("s", a))
        else:
            # fallback: sign-sort, |w3| fold, one mixed boundary chunk
            # reduced by a +-1 mask matvec
            perm = np.argsort(w3v <= 0, kind="stable")  # w3>0 units first
            npos = int((w3v > 0).sum())
            w3s = np.abs(w3v[perm])
            W2p[lvl] = W2f[lvl][:, perm] * w3s[None, :]
            b2p[lvl] = b2f[lvl][perm] * w3s
            cb, pb = divmod(npos, 128)
            mskf[lvl, :, 2] = 1.0 if cb > 0 else -1.0
            if pb > 0:
                mskf[lvl, :pb, 0] = 1.0
                mskf[lvl, pb:, 0] = -1.0
                modes.append(("m", cb))
            else:
                modes.append(("s", cb))  # chunk-aligned: pure chunks only
    modes = tuple(modes)

    # per-core padded index lists + validity masks
    idx = np.zeros((NC, r_core), dtype=np.int64)
    valid = np.zeros((NC, r_core), dtype=bool)
    lvl_start = np.concatenate([[0], np.cumsum(counts)])
    seg_off = 0
    for lvl in range(L):
        rows = order[lvl_start[lvl]:lvl_start[lvl + 1]]
        nl = len(rows)
        q, rem = divmod(nl, NC)
        start = 0
        for c in range(NC):
            cnt = q + (1 if c < rem else 0)
            idx[c, seg_off:seg_off + cnt] = rows[start:start + cnt]
            valid[c, seg_off:seg_off + cnt] = True
            start += cnt
        seg_off += caps[lvl]

    key = (tuple(caps), modes)
    nc = _PROGRAM_CACHE.get(key)
    if nc is None:
        nc = _build_program(caps, modes)
        _PROGRAM_CACHE[key] = nc

    W1b = W1f.astype(BF16_NP)
    W2b = W2p.astype(BF16_NP)
    in_maps = []
    for c in range(NC):
        xTc = np.ascontiguousarray(x[idx[c]].T)  # [D, r_core]
        in_maps.append({
            "xT": xTc.astype(BF16_NP),
            "W1": W1b,
            "W2": W2b,
            "b1": b1f,
            "b2": b2p,
            "b3": b3f,
            "msk": mskf.astype(BF16_NP),
        })

    del W2p
    trace = bool(os.environ.get("BASS_KERNEL_TRACE"))
    try:
        res = run_bass_kernel_spmd(nc, in_maps, core_ids=list(range(NC)), trace=trace)
    except Exception:
        # transient NRT_EXEC_UNIT_UNRECOVERABLE wedges have been observed to
        # clear on the next attempt
        import time
        time.sleep(5)
        res = run_bass_kernel_spmd(nc, in_maps, core_ids=list(range(NC)), trace=trace)
    LAST_RESULTS = res

    result = np.zeros((n, 1), dtype=np.float32)
    for c in range(NC):
        o = np.asarray(res.results[c]["out"]).reshape(-1)
        result[idx[c][valid[c]], 0] = o[valid[c]]
    return result
